# revision 24
# baseline (speedup 1.0000x reference)
"""Trainium2 Bass kernel: attention layer (out, score) sharded batch-parallel over 8 cores.

Math per batch element b (one NeuronCore each):
    q = (x @ Wq + bq) * 0.25 ; k = x @ Wk + bk ; v = x @ Wv + bv
    score[n,m,h] = sum_f k[n,f,h] q[m,f,h] + mask[n,m] - 1e10*pad[m]      -> output 1 (32MB/core)
    attn = softmax_m(score); o[n,f,h] = sum_m attn v; out = o @ Wo + bo   -> output 2

Engine plan per core:
  o1 (score out): PE K=16 matmuls [n,m] -> DVE fused (psum + maskcomb) into
     h-interleaved staging -> contiguous 32KB-row DMA to HBM.
  o2 (attention): PE transposed score [m,n] + identity-matmul accumulates the
     combined mask (incl. pad); ScalarE fused exp(psum) -> fp16; PE contracts
     with v (leading ones column = softmax denominator); normalization via
     gather/replicate matmuls; out-projection with zero-padded Wo groups.

Hardware constraints honored throughout:
  - compute-engine partition bases must be 32-aligned (heads live in 32-slots
    of two group tensors; denominator at slot+0, o values at slot+1..16);
  - PE matmuls and HWDGE DMAs carry at most ONE semaphore wait: all constant
    matmul operands are packed into one DRAM param, DMA'd once, and shadowed
    through a single ScalarE copy; a primer matmul covers the Pool clock.
"""

import numpy as np
from contextlib import ExitStack

import concourse.bass as bass
import concourse.bacc as bacc
import concourse.mybir as mybir
import concourse.tile as tile
from concourse.bass_utils import run_bass_kernel_spmd

F32 = mybir.dt.float32
F16 = mybir.dt.float16
U8 = mybir.dt.uint8
AF = mybir.ActivationFunctionType

B, N, D, H, HD = 8, 1024, 128, 8, 16
NT = N // 128
LARGE = 1.0e10

# packed constant layout (free-dim offsets in the [128, CW] consts param)
_OFF = {}
_cw = 0
for _name, _w in [
    ("ident", 128), ("wqg0", 128), ("wqg1", 128), ("wkg0", 128), ("wkg1", 128),
    ("wvp", 128), ("wog0", 128), ("wog1", 128), ("e4", 128), ("bvp", 128),
    ("onesr", 128), ("ones5", 512), ("bqr0", 128), ("bqr1", 128),
    ("bkr0", 128), ("bkr1", 128), ("bor", 128), ("misc", 32),
]:
    _OFF[_name] = _cw
    _cw += _w
CW = _cw
# Row-0 blocks: bvp, onesr, ones5 (512 wide), bqr*/bkr* (pre-scaled bias rows
# in slot layout), bor. e4 uses rows 0..3. misc: gsel one-hots (cols 0..3).
# All biases are applied as rank-1 ones-matmuls into PSUM, so no activation
# bias APs are needed (dodges per-instruction wait limits).


def build_nc():
    nc = bacc.Bacc()
    x_p = nc.declare_dram_parameter("x", [N, D], F32, isOutput=False)
    mask_p = nc.declare_dram_parameter("attn_mask", [N, N], F32, isOutput=False)
    maskT_p = nc.declare_dram_parameter("attn_maskT", [N, N], F32, isOutput=False)
    consts_p = nc.declare_dram_parameter("consts", [D, CW], F32, isOutput=False)
    score_p = nc.declare_dram_parameter("score", [N, N, H], F32, isOutput=True)
    out_p = nc.declare_dram_parameter("out", [N, D], F32, isOutput=True)

    with tile.TileContext(nc) as tc, ExitStack() as ctx:
        const = ctx.enter_context(tc.tile_pool(name="const", bufs=1))

        pjx = ExitStack()
        pj0 = pjx.enter_context(tc.tile_pool(name="pj0", bufs=1))
        consts = pj0.tile([128, CW], F32, tag="consts")
        nc.sync.dma_start(consts[:], consts_p[:, :])
        ca = const.tile([128, CW], F32, tag="ca")
        nc.scalar.copy(ca[:], consts[:])

        def C(name, w=128, rows=(0, 128)):
            return ca[rows[0] : rows[1], _OFF[name] : _OFF[name] + w]

        mo = _OFF["misc"]
        ident_a = C("ident")
        wq_a = [C("wqg0"), C("wqg1")]
        wk_a = [C("wkg0"), C("wkg1")]
        wv_a = C("wvp")
        wg_a = [C("wog0"), C("wog1")]
        e4_a = C("e4", rows=(0, 4))
        bv_a = C("bvp", rows=(0, 1))
        ones_a = C("onesr", rows=(0, 1))
        ones5_a = C("ones5", w=512, rows=(0, 1))
        bqr_a = [C("bqr0", rows=(0, 1)), C("bqr1", rows=(0, 1))]
        bkr_a = [C("bkr0", rows=(0, 1)), C("bkr1", rows=(0, 1))]
        bor_a = C("bor", rows=(0, 1))
        gsel_a = ca[:, mo : mo + 4]

        # maskc[p, t, m] = mask[t*128+p, m] - 1e10*pad[m] (combined on host);
        # serves o1 directly and o2 via PE transposes.
        maskc_all = const.tile([128, NT, N], F32, tag="maskc")
        nc.sync.dma_start(maskc_all[:], mask_p.rearrange("(t p) m -> p t m", p=128))
        maskcT_all = const.tile([128, NT, N], F32, tag="maskcT")
        nc.sync.dma_start(maskcT_all[:], maskT_p.rearrange("(t p) n -> p t n", p=128))

        # x resident + ACT shadow for the transposes
        x_all = pj0.tile([128, NT, D], F32, tag="xall")
        nc.sync.dma_start(x_all[:], x_p.rearrange("(t p) i -> p t i", p=128))
        xa_all = pj0.tile([128, NT, D], F32, tag="xaall")
        nc.scalar.copy(xa_all[:], x_all[:])

        xT = pj0.tile([128, N], F32, tag="xT")
        qTg, kTg = [], []
        for g in range(2):
            qT_t = const.tile([128, N], F32, tag=f"qT{g}")
            kT_t = const.tile([128, N], F32, tag=f"kT{g}")
            qTg.append(qT_t)
            kTg.append(kT_t)
        v_aug = const.tile([128, NT, H, 32], F16, tag="vaug")
        nc.gpsimd.memset(v_aug[:], 0.0)
        nc.gpsimd.memset(v_aug[:, :, :, 0:1], 1.0)
        expT_all = const.tile([128, H * N + 8], F16, tag="expTall")
        scr16 = const.tile([1, 4], F16, tag="scr16")
        scr32 = const.tile([1, 4], F32, tag="scr32")
        # wait-absorbers: raise ACT/DVE observed clocks past Pool/DMA producers
        # so downstream instructions carry a single semaphore wait each.
        nc.scalar.copy(scr16[0:1, 0:1], v_aug[0:1, 0, 0, 0:1])
        nc.vector.tensor_copy(scr32[0:1, 0:1], maskc_all[0:1, 0, 0:1])

        with tc.tile_pool(name="pj", bufs=2) as pj, tc.tile_pool(
            name="pjps", bufs=2, space="PSUM"
        ) as pjps:
            # Pool-clock primer: one PE op waiting on gpsimd (v_aug memsets);
            # later PE readers of Pool-written tiles then carry one wait.
            prim_ps = pjps.tile([128, 128], F32, tag="prim")            # x transpose via PE
            for t in range(NT):
                pt = pjps.tile([128, 128], F32, tag="pt")
                nc.tensor.transpose(pt[:], xa_all[:, t, :], ident_a[:])
                nc.scalar.copy(xT[:, t * 128 : (t + 1) * 128], pt[:])

            # q/k projections: psum[slot+f, m] = Wslot.T @ xT + bias, per group
            # (0.25 q-scale folded into wqg/bqr on host)
            for g in range(2):
                for half in range(2):
                    sl = slice(half * 512, (half + 1) * 512)
                    psq = pjps.tile([128, 512], F32, tag="pp")
                    nc.tensor.matmul(psq[:], wq_a[g][:], xT[:, sl], start=True, stop=False)
                    nc.tensor.matmul(psq[:], bqr_a[g][:], ones5_a[:], start=False, stop=True)
                    nc.scalar.copy(qTg[g][:, sl], psq[:])
                    psk = pjps.tile([128, 512], F32, tag="pp")
                    nc.tensor.matmul(psk[:], wk_a[g][:], xT[:, sl], start=True, stop=False)
                    nc.tensor.matmul(psk[:], bkr_a[g][:], ones5_a[:], start=False, stop=True)
                    nc.scalar.copy(kTg[g][:, sl], psk[:])

            # v projection (bias via rank-1 matmul), fp16, ones col at 0
            for t in range(NT):
                psv = pjps.tile([128, 128], F32, tag="psv")
                nc.tensor.matmul(
                    psv[:], xT[:, t * 128 : (t + 1) * 128], wv_a[:], start=True, stop=False
                )
                nc.tensor.matmul(psv[:], ones_a[:], bv_a[:], start=False, stop=True)
                nc.scalar.copy(
                    v_aug[:, t, :, 1:17],
                    psv[:].rearrange("p (h f) -> p h f", h=H),
                )

            # Pool-clock primer: one PE op waiting on gpsimd (v_aug memsets);
            # later PE readers of Pool-written tiles then carry one wait.
            prim_ps = pjps.tile([128, 128], F32, tag="prim")
            nc.tensor.matmul(
                prim_ps[0:32, 0:32],
                v_aug[:, 0, 0, :],
                v_aug[:, 0, 0, :],
                start=True,
                stop=True,
            )

        pjx.close()

        # ============ main loop ============
        stg_pool = ctx.enter_context(tc.tile_pool(name="stg", bufs=2))
        oTp = ctx.enter_context(tc.tile_pool(name="oTp", bufs=2, space="PSUM"))
        mainps = ExitStack()
        ps1 = mainps.enter_context(tc.tile_pool(name="ps1", bufs=2, space="PSUM"))
        ps2 = mainps.enter_context(tc.tile_pool(name="ps2", bufs=2, space="PSUM"))

        oT_ps = []
        for g in range(2):
            oT_g = oTp.tile([128, N], F32, tag="oT")
            oT_ps.append(oT_g)

        for i in range(NT):
            # ---- o1: score rows tile i ----
            nt = i
            stg = stg_pool.tile([128, N * H + 8], F32, tag="stg")
            stg3 = stg[:, 0 : N * H].rearrange("p (m h) -> p m h", h=H)
            # touch B: first writer of the reused slot, absorbs the DMA-reader
            # release into one wait (its own-engine WAW is covered by touch A)
            nc.vector.memset(stg[0:1, 0:1], 0.0)
            for h in range(H):
                g, slot = h // 4, (h % 4) * 32
                hs = slice(slot, slot + HD)
                for mh in range(2):
                    msl = slice(mh * 512, (mh + 1) * 512)
                    ps = ps1.tile([128, 512], F32, tag="s1")
                    nc.tensor.matmul(
                        ps[:],
                        kTg[g][hs, nt * 128 : (nt + 1) * 128],
                        qTg[g][hs, msl],
                        start=True,
                        stop=True,
                        tile_position=(slot, 0),
                    )
                    nc.vector.tensor_add(stg3[:, msl, h], ps[:], maskc_all[:, nt, msl])
            # touch A: last DVE write in the pad region; chains slot WAW so the
            # next tile's touch B sees a single semaphore
            nc.vector.memset(stg[0:1, N * H : N * H + 1], 0.0)
            nc.sync.dma_start(
                score_p[nt * 128 : (nt + 1) * 128, :, :],
                stg[:, 0 : N * H].rearrange("p (m h) -> p m h", h=H),
            )

            # ---- o2: m-chunk i ----
            mc = i
            # touch B/A pair for the statically-allocated expT tensor
            nc.scalar.copy(expT_all[0:1, 0:1], scr16[0:1, 0:1])
            for h in range(H):
                g, slot = h // 4, (h % 4) * 32
                hs = slice(slot, slot + HD)
                eo = h * N
                for nh in range(2):
                    nsl = slice(nh * 512, (nh + 1) * 512)
                    ps = ps2.tile([128, 512], F32, tag="s2")
                    nc.tensor.matmul(
                        ps[:],
                        qTg[g][hs, mc * 128 : (mc + 1) * 128],
                        kTg[g][hs, nsl],
                        start=True,
                        stop=False,
                        tile_position=(slot, 0),
                    )
                    nc.tensor.matmul(
                        ps[:], ident_a[:], maskcT_all[:, mc, nsl], start=False, stop=True
                    )
                    nc.scalar.activation(
                        expT_all[:, eo + nh * 512 : eo + (nh + 1) * 512], ps[:], AF.Exp
                    )
                for nh in range(2):
                    nsl = slice(nh * 512, (nh + 1) * 512)
                    nc.tensor.matmul(
                        oT_ps[g][slot : slot + 32, nsl],
                        v_aug[:, mc, h, :],
                        expT_all[:, eo + nh * 512 : eo + (nh + 1) * 512],
                        start=(mc == 0),
                        stop=(mc == NT - 1),
                        tile_position=(0, slot),
                    )
            nc.scalar.copy(expT_all[0:1, H * N : H * N + 1], scr16[0:1, 0:1])

        # ============ tail: normalize + out-projection ============
        mainps.close()
        with tc.tile_pool(name="tl", bufs=1) as tl, tc.tile_pool(
            name="tlps", bufs=1, space="PSUM"
        ) as tlps, tc.tile_pool(name="tlo", bufs=1) as tlo:
            oT_sb = []
            for g in range(2):
                oTsb_g = tl.tile([128, N], F32, tag=f"oTsb{g}")
                oT_sb.append(oTsb_g)
            for g in range(2):
                nc.scalar.copy(oT_sb[g][:], oT_ps[g][:])

            oTn_a = []
            for g in range(2):
                oTna_g = tl.tile([128, N], F32, tag=f"oTna{g}")
                oTn_a.append(oTna_g)
            for g in range(2):
                for nh in range(2):
                    nsl = slice(nh * 512, (nh + 1) * 512)
                    dn = tlps.tile([128, 512], F32, tag="dn")
                    nc.tensor.matmul(
                        dn[0:4, :], gsel_a[:], oT_sb[g][:, nsl], start=True, stop=True
                    )
                    dn_a = tlo.tile([4, 512], F32, tag="dna")
                    nc.scalar.copy(dn_a[:], dn[0:4, :])
                    rden = tlo.tile([4, 512], F32, tag="rden")
                    nc.vector.reciprocal(rden[:], dn_a[:])
                    rden_a = tlo.tile([4, 512], F32, tag="dna")
                    nc.scalar.copy(rden_a[:], rden[:])
                    rep = tlps.tile([128, 512], F32, tag="rep")
                    nc.tensor.matmul(rep[:], e4_a[:], rden_a[:], start=True, stop=True)
                    rep_a = tlo.tile([128, 512], F32, tag="repa")
                    nc.scalar.copy(rep_a[:], rep[:])
                    oTn = tlo.tile([128, 512], F32, tag="oTn")
                    nc.vector.tensor_mul(oTn[:], oT_sb[g][:, nsl], rep_a[:])
                    nc.scalar.copy(oTn_a[g][:, nsl], oTn[:])

            outT = tl.tile([128, N], F32, tag="outT")
            for nh in range(2):
                nsl = slice(nh * 512, (nh + 1) * 512)
                pso = tlps.tile([128, 512], F32, tag="pso")
                nc.tensor.matmul(pso[:], wg_a[0][:], oTn_a[0][:, nsl], start=True, stop=False)
                nc.tensor.matmul(pso[:], wg_a[1][:], oTn_a[1][:, nsl], start=False, stop=False)
                nc.tensor.matmul(pso[:], bor_a[:], ones5_a[:], start=False, stop=True)
                nc.scalar.copy(outT[:, nsl], pso[:])

            out_sb = tl.tile([128, NT, D], F32, tag="outsb")
            for t in range(NT):
                pst = tlps.tile([128, 512], F32, tag="pst")
                nc.tensor.transpose(pst[:, 0:128], outT[:, t * 128 : (t + 1) * 128], ident_a[:])
                nc.scalar.copy(out_sb[:, t, :], pst[:, 0:128])
            nc.sync.dma_start(out_p.rearrange("(t p) j -> p t j", p=128), out_sb[:])

    nc.compile()
    return nc


def _pack_consts(Wq, bq, Wk, bk, Wv, bv, Wo, bo):
    c = np.zeros((D, CW), np.float32)
    c[:, _OFF["ident"] : _OFF["ident"] + D] = np.eye(D, dtype=np.float32)
    mo = _OFF["misc"]
    for g in range(2):
        wq_g = np.zeros((D, D), np.float32)
        wk_g = np.zeros((D, D), np.float32)
        wo_g = np.zeros((D, D), np.float32)
        for hh in range(4):
            h = g * 4 + hh
            cols = [f * H + h for f in range(HD)]
            wq_g[:, hh * 32 : hh * 32 + HD] = 0.25 * Wq[:, cols]
            wk_g[:, hh * 32 : hh * 32 + HD] = Wk[:, cols]
            wo_g[hh * 32 + 1 : hh * 32 + 1 + HD, :] = Wo[cols, :]
            c[0, _OFF[f"bqr{g}"] + hh * 32 : _OFF[f"bqr{g}"] + hh * 32 + HD] = 0.25 * bq[cols]
            c[0, _OFF[f"bkr{g}"] + hh * 32 : _OFF[f"bkr{g}"] + hh * 32 + HD] = bk[cols]
        c[:, _OFF[f"wqg{g}"] : _OFF[f"wqg{g}"] + D] = wq_g
        c[:, _OFF[f"wkg{g}"] : _OFF[f"wkg{g}"] + D] = wk_g
        c[:, _OFF[f"wog{g}"] : _OFF[f"wog{g}"] + D] = wo_g
    perm = [f * H + h for h in range(H) for f in range(HD)]
    c[:, _OFF["wvp"] : _OFF["wvp"] + D] = Wv[:, perm]
    eo = _OFF["e4"]
    for hh in range(4):
        c[hh, eo + hh * 32 : eo + (hh + 1) * 32] = 1.0
        c[hh * 32, mo + hh] = 1.0  # gsel one-hots
    c[0, _OFF["bvp"] : _OFF["bvp"] + D] = bv[perm]
    c[0, _OFF["onesr"] : _OFF["onesr"] + D] = 1.0
    c[0, _OFF["ones5"] : _OFF["ones5"] + 512] = 1.0
    c[0, _OFF["bor"] : _OFF["bor"] + D] = bo
    return c


def make_in_maps(x, key_pad, attn_mask, Wq, bq, Wk, bk, Wv, bv, Wo, bo):
    x = np.asarray(x, dtype=np.float32)
    key_pad = np.asarray(key_pad)
    attn_mask = np.asarray(attn_mask, dtype=np.float32)
    consts = _pack_consts(
        np.asarray(Wq, np.float32), np.asarray(bq, np.float32),
        np.asarray(Wk, np.float32), np.asarray(bk, np.float32),
        np.asarray(Wv, np.float32), np.asarray(bv, np.float32),
        np.asarray(Wo, np.float32), np.asarray(bo, np.float32),
    )
    pad = key_pad.astype(np.float32) * np.float32(LARGE)
    in_maps = []
    for b in range(B):
        maskc = attn_mask[b].reshape(N, N) - pad[b][None, :]
        m = {
            "consts": consts,
            "x": np.ascontiguousarray(x[b]),
            "attn_mask": np.ascontiguousarray(maskc),
            "attn_maskT": np.ascontiguousarray(maskc.T),
        }
        in_maps.append(m)
    return in_maps


_NC_CACHE = None


def _get_nc():
    global _NC_CACHE
    if _NC_CACHE is None:
        _NC_CACHE = build_nc()
    return _NC_CACHE


def kernel(x, key_pad, attn_mask, Wq, bq, Wk, bk, Wv, bv, Wo, bo):
    nc = _get_nc()
    in_maps = make_in_maps(x, key_pad, attn_mask, Wq, bq, Wk, bk, Wv, bv, Wo, bo)
    res = run_bass_kernel_spmd(nc, in_maps, core_ids=list(range(B))).results
    out = np.stack([res[b]["out"] for b in range(B)], axis=0)
    score = np.stack([res[b]["score"] for b in range(B)], axis=0)
    return out, score
